# revision 11
# baseline (speedup 1.0000x reference)
"""Trainium2 Bass kernel for a GCN layer:
    out = segment_sum(edge_w * (x @ W.T)[edge_src], edge_dst)

Restructured as aggregate-then-transform (matmul commutes with the sum):
    agg = segment_sum(edge_w * x[edge_src], edge_dst);  out = agg @ W.T

Sharding: dst-node partition across 8 NeuronCores (core c owns dst rows
[c*12500, (c+1)*12500)). Host staging pre-gathers x rows per edge into
dense per-batch tiles (G) and pre-expands the edge weights into one-hot
scatter matrices (S), so the device kernel is a pure dense-streaming
SpMM: no gpsimd, no descriptor-generation bottleneck, all transfers at
HBM line rate.

Device pipeline per core:
  - dst windows of 64; PSUM bank [128,512] f32 = 8 windows; block =
    3 banks = 24 windows; 9 blocks (196 windows total).
  - per 128-edge batch b targeting window w:
      G[b] : [128 edges, 128 feat] bf16   (pre-gathered x rows)
      S[b] : [128 edges, 64 win]   bf16   (S[e, dst_rel(e)] = edge_w(e))
      psum[bank(w)][:, col(w)] += G[b]^T @ S[b]   (tensor engine)
  - tails per bank: psum -> bf16 aggT -> matmul with W^T -> f32 out.
  - batches per window are padded to the max count over cores so one
    SPMD-static program serves all 8 cores; pad slots have S rows = 0.
"""
import sys
sys.path.insert(0, "/opt/trn_rl_repo")

import numpy as np
import ml_dtypes
from contextlib import ExitStack

N_NODES = 100000
N_EDGES = 1600000
D = 128
N_CORES = 8
NPC = N_NODES // N_CORES          # 12500 dst nodes per core
WIN = 32                          # dst window width (S width / matmul N)
N_WIN = (NPC + WIN - 1) // WIN    # 391 windows (last = 20 dsts)
BANK_COLS = 512                   # psum bank free cols (f32)
WINS_PER_BANK = BANK_COLS // WIN  # 16
BANKS_PER_BLK = 3
WINS_PER_BLK = BANKS_PER_BLK * WINS_PER_BANK  # 48
N_BLK = (N_WIN + WINS_PER_BLK - 1) // WINS_PER_BLK  # 9
BATCH = 128
SB_SLOTS = 64                     # batches per streamed super-chunk

bf16 = ml_dtypes.bfloat16


# ---------------------------------------------------------------- host prep
def build_metadata(x, edge_src, edge_dst, edge_w):
    """Bucket edges by dst core/window, pad to a shared SPMD schedule, and
    pre-stage the gathered feature tiles (G) and scatter matrices (S)."""
    x_bf = np.ascontiguousarray(np.asarray(x, dtype=np.float32).astype(bf16))
    edge_src = np.asarray(edge_src).astype(np.int64)
    edge_dst = np.asarray(edge_dst).astype(np.int64)
    edge_w = np.asarray(edge_w, dtype=np.float32)

    core_of = edge_dst // NPC
    per_core = []
    counts = np.zeros((N_CORES, N_WIN), dtype=np.int64)
    for c in range(N_CORES):
        m = core_of == c
        es = edge_src[m]
        dl = edge_dst[m] - c * NPC
        ew = edge_w[m]
        win = dl // WIN
        order = np.argsort(win, kind="stable")
        es, dl, ew, win = es[order], dl[order], ew[order], win[order]
        np.add.at(counts[c], win, 1)
        per_core.append((es, dl, ew))

    cmax = counts.max(axis=0)
    nb = np.maximum((cmax + BATCH - 1) // BATCH, 1)      # batches per window
    batch_win = np.repeat(np.arange(N_WIN), nb)          # window of batch i
    NBTOT = int(nb.sum())
    batch_start = np.concatenate([[0], np.cumsum(nb)])   # first batch of win

    # per-(block, bank) first/last batch -> psum start/stop flags
    start_flag = np.zeros(NBTOT, dtype=bool)
    stop_flag = np.zeros(NBTOT, dtype=bool)
    seen_first = {}
    last_seen = {}
    for i in range(NBTOT):
        w = batch_win[i]
        key = (w // WINS_PER_BLK, (w % WINS_PER_BLK) // WINS_PER_BANK)
        if key not in seen_first:
            seen_first[key] = i
            start_flag[i] = True
        last_seen[key] = i
    for key, i in last_seen.items():
        stop_flag[i] = True

    core_arrays = []
    for c in range(N_CORES):
        es, dl, ew = per_core[c]
        n_e = len(es)
        # slot of each edge within its window's padded batch region
        win = dl // WIN
        first_e = np.concatenate([[0], np.cumsum(counts[c])])
        rank_in_win = np.arange(n_e) - first_e[win]
        flat_slot = batch_start[win] * BATCH + rank_in_win
        b_id = flat_slot // BATCH
        s_id = flat_slot % BATCH

        G = np.zeros((NBTOT, BATCH, D), dtype=bf16)
        G[b_id, s_id] = x_bf[es]
        S = np.zeros((NBTOT, BATCH, WIN), dtype=bf16)
        S[b_id, s_id, dl - win * WIN] = ew.astype(bf16)

        core_arrays.append(dict(
            g_all=np.ascontiguousarray(G.transpose(1, 0, 2)
                                       .reshape(BATCH, NBTOT * D)),
            s_all=np.ascontiguousarray(S.transpose(1, 0, 2)
                                       .reshape(BATCH, NBTOT * WIN))))

    meta = dict(NBTOT=NBTOT, batch_win=batch_win,
                start_flag=start_flag, stop_flag=stop_flag)
    return meta, core_arrays


# ------------------------------------------------------------- bass program
def build_program(meta):
    from concourse import bass, bacc, tile, mybir

    BF16 = mybir.dt.bfloat16
    F32 = mybir.dt.float32

    NBTOT = meta["NBTOT"]
    batch_win = meta["batch_win"]
    start_flag = meta["start_flag"]
    stop_flag = meta["stop_flag"]

    nc = bacc.Bacc(None)
    g_d = nc.declare_dram_parameter("g_all", [BATCH, NBTOT * D], BF16,
                                    isOutput=False)
    s_d = nc.declare_dram_parameter("s_all", [BATCH, NBTOT * WIN], BF16,
                                    isOutput=False)
    wt_d = nc.declare_dram_parameter("wt", [D, D], BF16, isOutput=False)
    out_d = nc.declare_dram_parameter("out", [NPC, D], BF16, isOutput=True)

    # batches of each block, chunked into super-chunks of SB_SLOTS
    blk_ranges = []
    for b in range(N_BLK):
        lo = int(np.searchsorted(batch_win, b * WINS_PER_BLK))
        hi = int(np.searchsorted(batch_win, (b + 1) * WINS_PER_BLK))
        blk_ranges.append((lo, hi))

    with tile.TileContext(nc) as tc, ExitStack() as ctx:
        const_pool = ctx.enter_context(tc.tile_pool(name="const", bufs=1))
        g_pool = ctx.enter_context(tc.tile_pool(name="gsb", bufs=4))
        s_pool = ctx.enter_context(tc.tile_pool(name="ssb", bufs=4))
        agg_pool = ctx.enter_context(tc.tile_pool(name="agg", bufs=3))
        o_pool = ctx.enter_context(tc.tile_pool(name="osb", bufs=4))
        psum_pool = ctx.enter_context(
            tc.tile_pool(name="psum", bufs=6, space="PSUM"))
        pout_pool = ctx.enter_context(
            tc.tile_pool(name="pout", bufs=2, space="PSUM"))

        wt_t = const_pool.tile([D, D], BF16, tag="wt")
        nc.sync.dma_start(wt_t[:], wt_d[:])

        for b in range(N_BLK):
            lo, hi = blk_ranges[b]
            nwin = min(WINS_PER_BLK, N_WIN - b * WINS_PER_BLK)
            nbank = (nwin + WINS_PER_BANK - 1) // WINS_PER_BANK
            bank_tiles = []
            for k in range(nbank):
                bank_tiles.append(psum_pool.tile(
                    [128, BANK_COLS], F32, tag="bank", name=f"bank_{b}_{k}"))
            for c0 in range(lo, hi, SB_SLOTS):
                nsl = min(SB_SLOTS, hi - c0)
                g_t = g_pool.tile([128, SB_SLOTS, D], BF16, tag="gt")
                nc.sync.dma_start(
                    g_t[:, :nsl, :],
                    g_d[:, c0 * D:(c0 + nsl) * D])
                s_t = s_pool.tile([128, SB_SLOTS, WIN], BF16, tag="st")
                nc.sync.dma_start(
                    s_t[:, :nsl, :],
                    s_d[:, c0 * WIN:(c0 + nsl) * WIN])
                for j in range(nsl):
                    bi = c0 + j
                    ww = int(batch_win[bi]) - b * WINS_PER_BLK
                    bank = ww // WINS_PER_BANK
                    col = (ww % WINS_PER_BANK) * WIN
                    nc.tensor.matmul(
                        bank_tiles[bank][:, col:col + WIN],
                        g_t[:, j, :],
                        s_t[:, j, :],
                        start=bool(start_flag[bi]),
                        stop=bool(stop_flag[bi]),
                        skip_group_check=True,
                    )
            blk_cols = min(NPC - b * WINS_PER_BLK * WIN, nwin * WIN)
            for k in range(nbank):
                cols_in_bank = min(BANK_COLS, blk_cols - k * BANK_COLS)
                agg_t = agg_pool.tile([128, BANK_COLS], BF16, tag="aggT")
                nc.vector.tensor_copy(agg_t[:, :cols_in_bank],
                                      bank_tiles[k][:, :cols_in_bank])
                for c0 in range(0, cols_in_bank, 128):
                    cw = min(128, cols_in_bank - c0)
                    pout = pout_pool.tile([128, D], F32, tag="pout")
                    nc.tensor.matmul(
                        pout[:cw, :], agg_t[:, c0:c0 + cw], wt_t[:, :],
                        start=True, stop=True, skip_group_check=True)
                    osb = o_pool.tile([128, D], BF16, tag="osb")
                    nc.scalar.copy(osb[:cw, :], pout[:cw, :])
                    r0 = b * WINS_PER_BLK * WIN + k * BANK_COLS + c0
                    nc.scalar.dma_start(out_d[r0:r0 + cw, :], osb[:cw, :])
    nc.finalize()
    return nc


# ------------------------------------------------------------------ runner
def kernel(**inputs):
    x = np.asarray(inputs["x"], dtype=np.float32)
    W = np.asarray(inputs["W"], dtype=np.float32)
    edge_src = np.asarray(inputs["edge_src"])
    edge_dst = np.asarray(inputs["edge_dst"])
    edge_w = np.asarray(inputs["edge_w"], dtype=np.float32)

    meta, arrs = build_metadata(x, edge_src, edge_dst, edge_w)
    nc = build_program(meta)

    wt_bf16 = np.ascontiguousarray(W.T.astype(bf16))
    in_maps = []
    for c in range(N_CORES):
        in_maps.append(dict(
            wt=wt_bf16,
            g_all=arrs[c]["g_all"],
            s_all=arrs[c]["s_all"]))

    from concourse.bass_utils import run_bass_kernel_spmd
    res = run_bass_kernel_spmd(nc, in_maps, list(range(N_CORES)))
    out = np.concatenate(
        [np.asarray(res.results[c]["out"]) for c in range(N_CORES)], axis=0)
    return out.astype(np.float32)


# revision 15
# speedup vs baseline: 1.1719x; 1.1719x over previous
"""Trainium2 Bass kernel for a GCN layer:
    out = segment_sum(edge_w * (x @ W.T)[edge_src], edge_dst)

Restructured as aggregate-then-transform (matmul commutes with the sum):
    agg = segment_sum(edge_w * x[edge_src], edge_dst);  out = agg @ W.T

Sharding: dst-node partition across 8 NeuronCores (core c owns dst rows
[c*12500, (c+1)*12500)). Host staging pre-gathers x rows per edge into
dense per-batch tiles (G) and pre-expands the edge weights into one-hot
scatter matrices (S), so the device kernel is a pure dense-streaming
SpMM: no gpsimd, no descriptor-generation bottleneck, all transfers at
HBM line rate.

Device pipeline per core:
  - dst windows of 64; PSUM bank [128,512] f32 = 8 windows; block =
    3 banks = 24 windows; 9 blocks (196 windows total).
  - per 128-edge batch b targeting window w:
      G[b] : [128 edges, 128 feat] bf16   (pre-gathered x rows)
      S[b] : [128 edges, 64 win]   bf16   (S[e, dst_rel(e)] = edge_w(e))
      psum[bank(w)][:, col(w)] += G[b]^T @ S[b]   (tensor engine)
  - tails per bank: psum -> bf16 aggT -> matmul with W^T -> f32 out.
  - batches per window are padded to the max count over cores so one
    SPMD-static program serves all 8 cores; pad slots have S rows = 0.
"""
import sys
sys.path.insert(0, "/opt/trn_rl_repo")

import numpy as np
import ml_dtypes
from contextlib import ExitStack

N_NODES = 100000
N_EDGES = 1600000
D = 128
N_CORES = 8
NPC = N_NODES // N_CORES          # 12500 dst nodes per core
WIN = 32                          # dst window width (S width / matmul N)
N_WIN = (NPC + WIN - 1) // WIN    # 391 windows (last = 20 dsts)
BANK_COLS = 512                   # psum bank free cols (f32)
WINS_PER_BANK = BANK_COLS // WIN  # 16
BANKS_PER_BLK = 3
WINS_PER_BLK = BANKS_PER_BLK * WINS_PER_BANK  # 48
N_BLK = (N_WIN + WINS_PER_BLK - 1) // WINS_PER_BLK  # 9
BATCH = 128
SB_SLOTS = 64                     # batches per streamed super-chunk

bf16 = ml_dtypes.bfloat16


# ---------------------------------------------------------------- host prep
def build_metadata(x, edge_src, edge_dst, edge_w):
    """Bucket edges by dst core/window, pad to a shared SPMD schedule, and
    pre-stage the gathered feature tiles (G) and scatter matrices (S)."""
    x_bf = np.ascontiguousarray(np.asarray(x, dtype=np.float32).astype(bf16))
    edge_src = np.asarray(edge_src).astype(np.int64)
    edge_dst = np.asarray(edge_dst).astype(np.int64)
    edge_w = np.asarray(edge_w, dtype=np.float32)

    core_of = edge_dst // NPC
    per_core = []
    counts = np.zeros((N_CORES, N_WIN), dtype=np.int64)
    for c in range(N_CORES):
        m = core_of == c
        es = edge_src[m]
        dl = edge_dst[m] - c * NPC
        ew = edge_w[m]
        win = dl // WIN
        order = np.argsort(win, kind="stable")
        es, dl, ew, win = es[order], dl[order], ew[order], win[order]
        np.add.at(counts[c], win, 1)
        per_core.append((es, dl, ew))

    cmax = counts.max(axis=0)
    nb = np.maximum((cmax + BATCH - 1) // BATCH, 1)      # batches per window
    batch_win = np.repeat(np.arange(N_WIN), nb)          # window of batch i
    NBTOT = int(nb.sum())
    batch_start = np.concatenate([[0], np.cumsum(nb)])   # first batch of win

    # per-(block, bank) first/last batch -> psum start/stop flags
    start_flag = np.zeros(NBTOT, dtype=bool)
    stop_flag = np.zeros(NBTOT, dtype=bool)
    seen_first = {}
    last_seen = {}
    for i in range(NBTOT):
        w = batch_win[i]
        key = (w // WINS_PER_BLK, (w % WINS_PER_BLK) // WINS_PER_BANK)
        if key not in seen_first:
            seen_first[key] = i
            start_flag[i] = True
        last_seen[key] = i
    for key, i in last_seen.items():
        stop_flag[i] = True

    core_arrays = []
    for c in range(N_CORES):
        es, dl, ew = per_core[c]
        n_e = len(es)
        # slot of each edge within its window's padded batch region
        win = dl // WIN
        first_e = np.concatenate([[0], np.cumsum(counts[c])])
        rank_in_win = np.arange(n_e) - first_e[win]
        flat_slot = batch_start[win] * BATCH + rank_in_win
        b_id = flat_slot // BATCH
        s_id = flat_slot % BATCH

        G = np.zeros((NBTOT, BATCH, D), dtype=bf16)
        G[b_id, s_id] = x_bf[es]
        S = np.zeros((NBTOT, BATCH, WIN), dtype=bf16)
        S[b_id, s_id, dl - win * WIN] = ew.astype(bf16)

        core_arrays.append(dict(
            g_all=np.ascontiguousarray(G.transpose(1, 0, 2)
                                       .reshape(BATCH, NBTOT * D)),
            s_all=np.ascontiguousarray(S.transpose(1, 0, 2)
                                       .reshape(BATCH, NBTOT * WIN))))

    meta = dict(NBTOT=NBTOT, batch_win=batch_win,
                start_flag=start_flag, stop_flag=stop_flag)
    return meta, core_arrays


# ------------------------------------------------------------- bass program
def build_program(meta):
    from concourse import bass, bacc, tile, mybir

    BF16 = mybir.dt.bfloat16
    F32 = mybir.dt.float32

    NBTOT = meta["NBTOT"]
    batch_win = meta["batch_win"]
    start_flag = meta["start_flag"]
    stop_flag = meta["stop_flag"]

    nc = bacc.Bacc(None)
    g_d = nc.declare_dram_parameter("g_all", [BATCH, NBTOT * D], BF16,
                                    isOutput=False)
    s_d = nc.declare_dram_parameter("s_all", [BATCH, NBTOT * WIN], BF16,
                                    isOutput=False)
    wt_d = nc.declare_dram_parameter("wt", [D, D], BF16, isOutput=False)
    # output stored chunk-major: out[p, c, f] = row c*128+p of the final
    # [NPC, D] result (host unscrambles); single end-of-kernel DMA keeps
    # compute-gated writes off the prefetch DMA sem lanes.
    n_chunks = (NPC + 127) // 128
    out_d = nc.declare_dram_parameter("out", [128, n_chunks * D], BF16,
                                      isOutput=True)

    # batches of each block, chunked into super-chunks of SB_SLOTS
    blk_ranges = []
    for b in range(N_BLK):
        lo = int(np.searchsorted(batch_win, b * WINS_PER_BLK))
        hi = int(np.searchsorted(batch_win, (b + 1) * WINS_PER_BLK))
        blk_ranges.append((lo, hi))

    with tile.TileContext(nc) as tc, ExitStack() as ctx:
        const_pool = ctx.enter_context(tc.tile_pool(name="const", bufs=1))
        g_pool = ctx.enter_context(tc.tile_pool(name="gsb", bufs=6))
        s_pool = ctx.enter_context(tc.tile_pool(name="ssb", bufs=6))
        agg_pool = ctx.enter_context(tc.tile_pool(name="agg", bufs=3))
        psum_pool = ctx.enter_context(
            tc.tile_pool(name="psum", bufs=6, space="PSUM"))
        pout_pool = ctx.enter_context(
            tc.tile_pool(name="pout", bufs=2, space="PSUM"))

        wt_t = const_pool.tile([D, D], BF16, tag="wt")
        nc.sync.dma_start(wt_t[:], wt_d[:])
        osb_all = const_pool.tile([128, n_chunks, D], BF16, tag="osb_all")

        for b in range(N_BLK):
            lo, hi = blk_ranges[b]
            nwin = min(WINS_PER_BLK, N_WIN - b * WINS_PER_BLK)
            nbank = (nwin + WINS_PER_BANK - 1) // WINS_PER_BANK
            bank_tiles = []
            for k in range(nbank):
                bank_tiles.append(psum_pool.tile(
                    [128, BANK_COLS], F32, tag="bank", name=f"bank_{b}_{k}"))
            for c0 in range(lo, hi, SB_SLOTS):
                nsl = min(SB_SLOTS, hi - c0)
                g_t = g_pool.tile([128, SB_SLOTS, D], BF16, tag="gt")
                nc.sync.dma_start(
                    g_t[:, :nsl, :],
                    g_d[:, c0 * D:(c0 + nsl) * D])
                s_t = s_pool.tile([128, SB_SLOTS, WIN], BF16, tag="st")
                nc.sync.dma_start(
                    s_t[:, :nsl, :],
                    s_d[:, c0 * WIN:(c0 + nsl) * WIN])
                for j in range(nsl):
                    bi = c0 + j
                    ww = int(batch_win[bi]) - b * WINS_PER_BLK
                    bank = ww // WINS_PER_BANK
                    col = (ww % WINS_PER_BANK) * WIN
                    nc.tensor.matmul(
                        bank_tiles[bank][:, col:col + WIN],
                        g_t[:, j, :],
                        s_t[:, j, :],
                        start=bool(start_flag[bi]),
                        stop=bool(stop_flag[bi]),
                        skip_group_check=True,
                    )
            blk_cols = min(NPC - b * WINS_PER_BLK * WIN, nwin * WIN)
            for k in range(nbank):
                cols_in_bank = min(BANK_COLS, blk_cols - k * BANK_COLS)
                agg_t = agg_pool.tile([128, BANK_COLS], BF16, tag="aggT")
                nc.vector.tensor_copy(agg_t[:, :cols_in_bank],
                                      bank_tiles[k][:, :cols_in_bank])
                for c0 in range(0, cols_in_bank, 128):
                    cw = min(128, cols_in_bank - c0)
                    pout = pout_pool.tile([128, D], F32, tag="pout")
                    nc.tensor.matmul(
                        pout[:cw, :], agg_t[:, c0:c0 + cw], wt_t[:, :],
                        start=True, stop=True, skip_group_check=True)
                    r0 = b * WINS_PER_BLK * WIN + k * BANK_COLS + c0
                    ci = r0 // 128
                    nc.scalar.copy(osb_all[:cw, ci, :], pout[:cw, :])
        nc.scalar.dma_start(out_d[:, :], osb_all[:, :, :])
    nc.finalize()
    return nc


# ------------------------------------------------------------------ runner
def kernel(**inputs):
    x = np.asarray(inputs["x"], dtype=np.float32)
    W = np.asarray(inputs["W"], dtype=np.float32)
    edge_src = np.asarray(inputs["edge_src"])
    edge_dst = np.asarray(inputs["edge_dst"])
    edge_w = np.asarray(inputs["edge_w"], dtype=np.float32)

    meta, arrs = build_metadata(x, edge_src, edge_dst, edge_w)
    nc = build_program(meta)

    wt_bf16 = np.ascontiguousarray(W.T.astype(bf16))
    in_maps = []
    for c in range(N_CORES):
        in_maps.append(dict(
            wt=wt_bf16,
            g_all=arrs[c]["g_all"],
            s_all=arrs[c]["s_all"]))

    from concourse.bass_utils import run_bass_kernel_spmd
    res = run_bass_kernel_spmd(nc, in_maps, list(range(N_CORES)))
    out = np.concatenate(
        [unscramble_out(np.asarray(res.results[c]["out"]))
         for c in range(N_CORES)], axis=0)
    return out.astype(np.float32)


def unscramble_out(raw):
    """[128, n_chunks*D] chunk-major device output -> [NPC, D] rows."""
    n_chunks = (NPC + 127) // 128
    return (raw.reshape(128, n_chunks, D).transpose(1, 0, 2)
            .reshape(n_chunks * 128, D)[:NPC])
